# revision 1
# baseline (speedup 1.0000x reference)
"""ConvTreeGRUCell on 8 Trainium2 NeuronCores.

Sharding: spatial over H. Each core owns 24 output rows (192/8) and
receives a 28-row input slab (2-row halo each side, zero-padded at the
image borders on the host). All three 3x3 convs, the per-child reset
gate, and the L-reductions are then fully local per core — no
collectives.

Per-core kernel (Tile framework):
  - frames are 28 rows x 194 cols (192 + zero pad col each side),
    flattened to 5432 elements + 1 front/back pad -> free size 5434.
    A 3x3 conv tap (dy,dx) is a single offset dy*194+dx into the flat
    frame; the zero pad columns absorb the row-wrap reads.
  - cat tiles [128, 5434]: partitions 0..63 = child_h[l] channels,
    64..127 = x channels (child first so every elementwise op on
    child/r/reset_hidden/child_sum shares partition base 0 — the
    walrus verifier requires identical partition ranges).  conv = 9 accumulating fp32r matmuls
    (K=128, M=64, N<=512 pixel windows) into one PSUM bank.
  - r_l = sigmoid(psum + br) on ScalarE (bias is per-partition).
  - reset_hidden accumulated on VectorE; child_h_sum on GPSIMD.
  - z/o convs run over [x | child_sum] and [x | reset_hidden] cat
    tiles; h = o + z*(child_sum - o).
"""

import os
import sys

import numpy as np

for _p in ("/opt/trn_rl_repo",):
    if _p not in sys.path and os.path.isdir(_p):
        sys.path.insert(0, _p)

import concourse.bass as bass
import concourse.tile as tile
from concourse import bacc
from concourse import mybir
from concourse.bass_utils import run_bass_kernel_spmd

F32 = mybir.dt.float32
F32R = mybir.dt.float32r

C = 64          # channels
L = 8           # children
HW = 192        # image H and W
NCORES = 8
OUT_ROWS = HW // NCORES          # 24 output rows per core
IN_ROWS = OUT_ROWS + 4           # 28-row slab (2-row halo each side)
WP = HW + 2                      # 194: padded row width
FRAME = IN_ROWS * WP             # 5432
FREE = FRAME + 2                 # 5434: +1 front pad, +1 tail pad
HALF = FREE // 2                 # 2717

# flat index of (row r, col c) in the frame = 1 + r*WP + c
# stage 1 (r gate / reset_hidden): output rows 1..26
S1_LO = 1 + 1 * WP               # 195
S1_HI = 1 + 26 * WP + 194       # 5239 (exclusive)
# stage 2 (z / o / h): output rows 2..25
S2_LO = 1 + 2 * WP               # 389
S2_HI = 1 + 25 * WP + 194       # 5045 (exclusive)

NWIN = 512

TAP_OFF = [dy * WP + dx for dy in (-1, 0, 1) for dx in (-1, 0, 1)]


def _windows(lo, hi):
    out = []
    s = lo
    while s < hi:
        out.append((s, min(NWIN, hi - s)))
        s += NWIN
    return out


S1WIN = _windows(S1_LO, S1_HI)
S2WIN = _windows(S2_LO, S2_HI)

_BUILT = None


def build_program():
    """Build the (SPMD, per-core) Bass program once."""
    nc = bacc.Bacc("TRN2")

    xin = nc.dram_tensor("xin", [C, FREE], F32R, kind="ExternalInput")
    cin = nc.dram_tensor("cin", [L, C, FREE], F32R, kind="ExternalInput")
    wrt = nc.dram_tensor("wrt", [2 * C, 9, C], F32R, kind="ExternalInput")
    wzt = nc.dram_tensor("wzt", [2 * C, 9, C], F32R, kind="ExternalInput")
    wot = nc.dram_tensor("wot", [2 * C, 9, C], F32R, kind="ExternalInput")
    brt = nc.dram_tensor("brt", [C, 1], F32, kind="ExternalInput")
    bzt = nc.dram_tensor("bzt", [C, 1], F32, kind="ExternalInput")
    bot = nc.dram_tensor("bot", [C, 1], F32, kind="ExternalInput")
    hout = nc.dram_tensor("hout", [C, OUT_ROWS, HW], F32, kind="ExternalOutput")

    with tile.TileContext(nc) as tc:
        with (
            tc.tile_pool(name="singles", bufs=1) as singles,
            tc.tile_pool(name="cats", bufs=1) as cats,
            tc.tile_pool(name="rbig", bufs=2) as rbig_pool,
            tc.tile_pool(name="tbig", bufs=1) as tbig_pool,
            tc.tile_pool(name="hwin", bufs=3) as hwin_pool,
            tc.tile_pool(name="s2", bufs=1) as s2_pool,
            tc.tile_pool(name="psum", bufs=6, space="PSUM") as psum_pool,
        ):
            # ---- persistent tiles ----
            xsrc = singles.tile([2 * C, HALF], F32R, tag="xsrc")
            wr = singles.tile([2 * C, 9, C], F32R, tag="wr")
            wz = singles.tile([2 * C, 9, C], F32R, tag="wz")
            wo = singles.tile([2 * C, 9, C], F32R, tag="wo")
            br = singles.tile([C, 1], F32, tag="br")
            bz = singles.tile([C, 1], F32, tag="bz")
            bo = singles.tile([C, 1], F32, tag="bo")
            zs = singles.tile([2 * C, FREE], F32R, tag="zs")    # [x | child_sum]
            orh = singles.tile([2 * C, FREE], F32R, tag="orh")  # [x | reset_hidden]

            # ---- loads: critical path first (child0 + catA x + Wr) ----
            catA = cats.tile([2 * C, FREE], F32R, tag="catA")
            catB = cats.tile([2 * C, FREE], F32R, tag="catB")
            nc.sync.dma_start(out=xsrc[0:C, :], in_=xin[:, 0:HALF])
            nc.sync.dma_start(out=xsrc[C:2 * C, :], in_=xin[:, HALF:FREE])
            nc.sync.dma_start(out=wr, in_=wrt[:])
            nc.sync.dma_start(out=br, in_=brt[:])
            nc.sync.dma_start(out=catA[0:C, :], in_=cin[0])
            nc.sync.dma_start(out=catA[C:2 * C, 0:HALF], in_=xsrc[0:C, :])
            nc.sync.dma_start(out=catA[C:2 * C, HALF:FREE], in_=xsrc[C:2 * C, :])
            nc.sync.dma_start(out=catB[0:C, :], in_=cin[1])
            nc.sync.dma_start(out=catB[C:2 * C, 0:HALF], in_=xsrc[0:C, :])
            nc.sync.dma_start(out=catB[C:2 * C, HALF:FREE], in_=xsrc[C:2 * C, :])
            nc.sync.dma_start(out=wz, in_=wzt[:])
            nc.sync.dma_start(out=wo, in_=wot[:])
            nc.sync.dma_start(out=bz, in_=bzt[:])
            nc.sync.dma_start(out=bo, in_=bot[:])
            for dst in (zs, orh):
                nc.sync.dma_start(out=dst[C:2 * C, 0:HALF], in_=xsrc[0:C, :])
                nc.sync.dma_start(out=dst[C:2 * C, HALF:FREE], in_=xsrc[C:2 * C, :])
            # reset_hidden rows 0 and 27 (and the flat pads) are never
            # written by the accumulation below but are read by the o-conv.
            # memset can't produce f32r; bounce zeros through a f32 scratch.
            zpad = singles.tile([C, S1_LO], F32, tag="zpad")
            nc.vector.memset(zpad, 0.0)
            nc.vector.tensor_copy(out=orh[0:C, 0:S1_LO], in_=zpad)
            nc.vector.tensor_copy(out=orh[0:C, S1_HI:FREE], in_=zpad[:, 0:FREE - S1_HI])

            # ---- stage 1: per-child reset gate + reductions ----
            for l in range(L):
                cat = catA if l % 2 == 0 else catB
                if l >= 2:
                    nc.sync.dma_start(out=cat[0:C, :], in_=cin[l])

                # child_h_sum accumulation on GPSIMD
                if l == 0:
                    nc.gpsimd.tensor_copy(
                        out=zs[0:C, :], in_=cat[0:C, :].bitcast(F32)
                    )
                else:
                    nc.gpsimd.tensor_add(
                        out=zs[0:C, :],
                        in0=zs[0:C, :].bitcast(F32),
                        in1=cat[0:C, :].bitcast(F32),
                    )

                rb = rbig_pool.tile([C, S1_HI - S1_LO], F32, tag="rb")
                for s, n in S1WIN:
                    ps = psum_pool.tile([C, NWIN], F32, tag="ps")
                    for t in range(9):
                        o = TAP_OFF[t]
                        nc.tensor.matmul(
                            out=ps[:, :n],
                            lhsT=wr[:, t, :],
                            rhs=cat[:, s + o:s + o + n],
                            start=(t == 0),
                            stop=(t == 8),
                        )
                    nc.scalar.activation(
                        out=rb[:, s - S1_LO:s - S1_LO + n],
                        in_=ps[:, :n],
                        func=mybir.ActivationFunctionType.Sigmoid,
                        bias=br[:, 0:1],
                    )
                if l == 0:
                    nc.vector.tensor_mul(
                        out=orh[0:C, S1_LO:S1_HI],
                        in0=rb,
                        in1=cat[0:C, S1_LO:S1_HI].bitcast(F32),
                    )
                else:
                    tb = tbig_pool.tile([C, S1_HI - S1_LO], F32, tag="tb")
                    nc.vector.tensor_mul(
                        out=tb,
                        in0=rb,
                        in1=cat[0:C, S1_LO:S1_HI].bitcast(F32),
                    )
                    nc.vector.tensor_add(
                        out=orh[0:C, S1_LO:S1_HI],
                        in0=orh[0:C, S1_LO:S1_HI].bitcast(F32),
                        in1=tb,
                    )

            # ---- stage 2: all z-conv windows first (only needs zs),
            # then o-conv windows with h fused per window ----
            S2N = S2_HI - S2_LO
            zb = s2_pool.tile([C, S2N], F32, tag="zb")
            ob = s2_pool.tile([C, S2N], F32, tag="ob")
            for wi, (s, n) in enumerate(S2WIN):
                psz = psum_pool.tile([C, NWIN], F32, tag="ps")
                for t in range(9):
                    o = TAP_OFF[t]
                    nc.tensor.matmul(
                        out=psz[:, :n],
                        lhsT=wz[:, t, :],
                        rhs=zs[:, s + o:s + o + n],
                        start=(t == 0),
                        stop=(t == 8),
                    )
                nc.scalar.activation(
                    out=zb[:, s - S2_LO:s - S2_LO + n],
                    in_=psz[:, :n],
                    func=mybir.ActivationFunctionType.Sigmoid,
                    bias=bz[:, 0:1],
                )

            for wi, (s, n) in enumerate(S2WIN):
                j = s - S2_LO
                pso = psum_pool.tile([C, NWIN], F32, tag="ps")
                for t in range(9):
                    o = TAP_OFF[t]
                    nc.tensor.matmul(
                        out=pso[:, :n],
                        lhsT=wo[:, t, :],
                        rhs=orh[:, s + o:s + o + n],
                        start=(t == 0),
                        stop=(t == 8),
                    )
                nc.scalar.activation(
                    out=ob[:, j:j + n],
                    in_=pso[:, :n],
                    func=mybir.ActivationFunctionType.Tanh,
                    bias=bo[:, 0:1],
                )
                # h_w = o_w + z_w * (child_sum_w - o_w), in place in ob
                t1w = hwin_pool.tile([C, NWIN], F32, tag="t1w")
                nc.vector.scalar_tensor_tensor(
                    out=t1w[:, :n],
                    in0=ob[:, j:j + n],
                    scalar=-1.0,
                    in1=zs[0:C, s:s + n].bitcast(F32),
                    op0=mybir.AluOpType.mult,
                    op1=mybir.AluOpType.add,
                )
                nc.vector.tensor_mul(
                    out=t1w[:, :n], in0=zb[:, j:j + n], in1=t1w[:, :n]
                )
                nc.vector.tensor_add(
                    out=ob[:, j:j + n], in0=ob[:, j:j + n], in1=t1w[:, :n]
                )

            # ---- store: drop the pad column of each row ----
            htr = ob.rearrange("p (r w) -> p r w", w=WP)
            nc.sync.dma_start(out=hout[:], in_=htr[:, 0:OUT_ROWS, 1:HW + 1])

    nc.finalize()
    return nc


def _get_program():
    global _BUILT
    if _BUILT is None:
        _BUILT = build_program()
    return _BUILT


def make_in_maps(x, child_h, Wr, br, Wz, bz, Wo, bo):
    """Host-side sharding: pad borders/columns and slice 28-row slabs."""
    x = np.asarray(x, dtype=np.float32)
    child_h = np.asarray(child_h, dtype=np.float32)

    # zero-pad H by 2 (halo at image border) and W by 1 (conv column pad)
    xp = np.zeros((C, HW + 4, WP), dtype=np.float32)
    xp[:, 2:2 + HW, 1:1 + HW] = x[0]
    cp = np.zeros((L, C, HW + 4, WP), dtype=np.float32)
    cp[:, :, 2:2 + HW, 1:1 + HW] = child_h[:, 0]

    def frame(a):  # [..., IN_ROWS, WP] -> [..., FREE] with 1-elem front/tail pad
        flat = a.reshape(a.shape[:-2] + (FRAME,))
        out = np.zeros(a.shape[:-2] + (FREE,), dtype=np.float32)
        out[..., 1:1 + FRAME] = flat
        return out

    def wprep(w):  # [C, 2C, 3, 3] -> [2C, 9, C] lhsT per tap
        wt = np.transpose(np.asarray(w, np.float32), (1, 2, 3, 0)).reshape(2 * C, 9, C)
        # cat layout is [child | x], reference weight rows are [x | child]
        return np.ascontiguousarray(np.concatenate([wt[C:], wt[:C]], axis=0))

    wrt, wzt, wot = wprep(Wr), wprep(Wz), wprep(Wo)
    brt = np.asarray(br, np.float32).reshape(C, 1)
    bzt = np.asarray(bz, np.float32).reshape(C, 1)
    bot = np.asarray(bo, np.float32).reshape(C, 1)

    in_maps = []
    for k in range(NCORES):
        r0 = k * OUT_ROWS  # global output row start; slab = rows r0-2 .. r0+26
        in_maps.append({
            "xin": frame(xp[:, r0:r0 + IN_ROWS, :]),
            "cin": frame(cp[:, :, r0:r0 + IN_ROWS, :]),
            "wrt": wrt, "wzt": wzt, "wot": wot,
            "brt": brt, "bzt": bzt, "bot": bot,
        })
    return in_maps


def run(in_maps, trace=False):
    nc = _get_program()
    return run_bass_kernel_spmd(nc, in_maps, list(range(NCORES)), trace=trace)


def kernel(x, child_h, Wr, br, Wz, bz, Wo, bo):
    in_maps = make_in_maps(x, child_h, Wr, br, Wz, bz, Wo, bo)
    res = run(in_maps).results
    out = np.empty((1, C, HW, HW), dtype=np.float32)
    for k in range(NCORES):
        out[0, :, k * OUT_ROWS:(k + 1) * OUT_ROWS, :] = res[k]["hout"]
    return out



# revision 6
# speedup vs baseline: 1.4258x; 1.4258x over previous
"""ConvTreeGRUCell on 8 Trainium2 NeuronCores.

Sharding: spatial over H. Each core owns 24 output rows (192/8) and
receives a 28-row input slab (2-row halo each side, zero-padded at the
image borders on the host). All convs and L-reductions are local.

v1 restructure (vs fp32r baseline):
  - everything bf16 on the matmul/vector path (tolerance is 2e-2).
  - reset gate conv is split: Wr = [Wr_x | Wr_c].  xr2 = Wr_x*x + br is
    computed ONCE with lhsT [Wr_x | Wr_x] (K=64 -> M=128, both output
    halves identical).  Children are then processed in PAIRS with a
    block-diagonal lhsT [[Wr_c,0],[0,Wr_c]] (K=128, M=128) over rhs
    [child_even; child_odd]; an identity matmul injects xr2 into the
    same PSUM accumulation.  5 matmuls/children-pair/window vs 9 per
    child before -> stage-1 columns drop ~1.8x.
  - child_sum and reset_hidden cross-partition (128->64) folds are done
    with an [I;I] matmul instead of GPSIMD/Vector loops; the pairwise
    sums/products stay on Vector at 128-partition bf16 rate.
  - z/o convs stay K=128 over [csum|x] and [rh|x]; h combined per
    window and stored per window.
"""

import os
import sys

import numpy as np
import ml_dtypes

for _p in ("/opt/trn_rl_repo",):
    if _p not in sys.path and os.path.isdir(_p):
        sys.path.insert(0, _p)

import concourse.bass as bass
import concourse.tile as tile
from concourse import bacc
from concourse import mybir
from concourse.bass_utils import run_bass_kernel_spmd

F32 = mybir.dt.float32
BF16 = mybir.dt.bfloat16
NPBF16 = ml_dtypes.bfloat16

C = 64          # channels
L = 8           # children
HW = 192        # image H and W
NCORES = 8
OUT_ROWS = HW // NCORES          # 24 output rows per core
IN_ROWS = OUT_ROWS + 4           # 28-row slab (2-row halo each side)
WP = HW + 2                      # 194: padded row width
FRAME = IN_ROWS * WP             # 5432
FREE = FRAME + 2                 # 5434: +1 front pad, +1 tail pad

# flat index of (row r, col c) in the frame = 1 + r*WP + c
S1_LO = 1 + 1 * WP               # 195   (r rows 1..26)
S1_HI = 1 + 26 * WP + 194        # 5239 (exclusive)
S1N = S1_HI - S1_LO              # 5044
S2_LO = 1 + 2 * WP               # 389   (h rows 2..25)
S2_HI = 1 + 25 * WP + 194        # 5045 (exclusive)
S2N = S2_HI - S2_LO              # 4656

NWIN = 512

TAP_OFF = [dy * WP + dx for dy in (-1, 0, 1) for dx in (-1, 0, 1)]


def _windows(lo, hi):
    out = []
    s = lo
    while s < hi:
        out.append((s, min(NWIN, hi - s)))
        s += NWIN
    return out


S1WIN = _windows(S1_LO, S1_HI)
S2WIN = _windows(S2_LO, S2_HI)

_BUILT = None


def build_program():
    nc = bacc.Bacc("TRN2")

    xin = nc.dram_tensor("xin", [C, FREE], BF16, kind="ExternalInput")
    cin = nc.dram_tensor("cin", [L, C, FREE], BF16, kind="ExternalInput")
    wrxt = nc.dram_tensor("wrxt", [2 * C, 9, 2 * C], BF16, kind="ExternalInput")
    wrct = nc.dram_tensor("wrct", [2 * C, 9, 2 * C], BF16, kind="ExternalInput")
    wzt = nc.dram_tensor("wzt", [2 * C, 9, C], BF16, kind="ExternalInput")
    wot = nc.dram_tensor("wot", [2 * C, 9, C], BF16, kind="ExternalInput")
    idt = nc.dram_tensor("idt", [2 * C, 2 * C], BF16, kind="ExternalInput")
    idvt = nc.dram_tensor("idvt", [2 * C, C], BF16, kind="ExternalInput")
    brt = nc.dram_tensor("brt", [2 * C, 1], F32, kind="ExternalInput")
    bzt = nc.dram_tensor("bzt", [C, 1], F32, kind="ExternalInput")
    bot = nc.dram_tensor("bot", [C, 1], F32, kind="ExternalInput")
    hout = nc.dram_tensor("hout", [C, S2N], BF16, kind="ExternalOutput")

    ID = mybir.ActivationFunctionType.Identity
    SIG = mybir.ActivationFunctionType.Sigmoid
    TANH = mybir.ActivationFunctionType.Tanh
    CP = mybir.ActivationFunctionType.Copy

    with tile.TileContext(nc) as tc:
        with (
            tc.tile_pool(name="singles", bufs=1) as singles,
            tc.tile_pool(name="cats", bufs=1) as cats,
            tc.tile_pool(name="rbp", bufs=2) as rb_pool,
            tc.tile_pool(name="rcp", bufs=2) as rc_pool,
            tc.tile_pool(name="hwp", bufs=3) as hw_pool,
            tc.tile_pool(name="psum", bufs=4, space="PSUM") as psum_pool,
            tc.tile_pool(name="psumf", bufs=4, space="PSUM") as psumf_pool,
        ):
            # ---- persistent tiles ----
            x128 = singles.tile([2 * C, FREE], BF16, tag="x128")
            wrx = singles.tile([2 * C, 9, 2 * C], BF16, tag="wrx")
            wrc = singles.tile([2 * C, 9, 2 * C], BF16, tag="wrc")
            wz = singles.tile([2 * C, 9, C], BF16, tag="wz")
            wo = singles.tile([2 * C, 9, C], BF16, tag="wo")
            i2h = singles.tile([2 * C, 2 * C], BF16, tag="i2h")
            i2v = singles.tile([2 * C, C], BF16, tag="i2v")
            br = singles.tile([2 * C, 1], F32, tag="br")
            bz = singles.tile([C, 1], F32, tag="bz")
            bo = singles.tile([C, 1], F32, tag="bo")
            xr2 = singles.tile([2 * C, S1N], BF16, tag="xr2")
            zs = singles.tile([2 * C, FREE], BF16, tag="zs")    # [csum | x]
            orh = singles.tile([2 * C, FREE], BF16, tag="orh")  # [rh | x]
            S = singles.tile([2 * C, FREE], BF16, tag="S")      # pair sums
            T = singles.tile([2 * C, S1N], BF16, tag="T")       # r*child sums
            zb = singles.tile([C, S2N], BF16, tag="zb")
            ob = singles.tile([C, S2N], BF16, tag="ob")

            # ---- loads: P0 critical path first ----
            nc.sync.dma_start(out=x128[0:C, :], in_=xin[:])
            nc.sync.dma_start(out=x128[C:2 * C, :], in_=xin[:])
            nc.sync.dma_start(out=wrx, in_=wrxt[:])
            nc.sync.dma_start(out=br, in_=brt[:])
            catt = []
            for p in range(4):
                cat = cats.tile([2 * C, FREE], BF16, tag=f"cat{p}")
                nc.sync.dma_start(out=cat[0:C, :], in_=cin[2 * p])
                nc.sync.dma_start(out=cat[C:2 * C, :], in_=cin[2 * p + 1])
                catt.append(cat)
            nc.sync.dma_start(out=wrc, in_=wrct[:])
            nc.sync.dma_start(out=i2h, in_=idt[:])
            nc.sync.dma_start(out=i2v, in_=idvt[:])
            nc.sync.dma_start(out=wz, in_=wzt[:])
            nc.sync.dma_start(out=wo, in_=wot[:])
            nc.sync.dma_start(out=bz, in_=bzt[:])
            nc.sync.dma_start(out=bo, in_=bot[:])
            nc.sync.dma_start(out=zs[C:2 * C, :], in_=xin[:])
            nc.sync.dma_start(out=orh[C:2 * C, :], in_=xin[:])
            # zero the csum/rh halves (pad cols outside S1 must be 0)
            nc.scalar.memzero(zs[0:C, :])
            nc.scalar.memzero(orh[0:C, :])

            # ---- P0: xr2 = [Wr_x*x + br] replicated on both halves ----
            for s, n in S1WIN:
                j = s - S1_LO
                ps = psum_pool.tile([2 * C, NWIN], F32, tag="ps")
                for t in range(9):
                    o = TAP_OFF[t]
                    nc.tensor.matmul(
                        out=ps[:, :n],
                        lhsT=wrx[:, t, :],
                        rhs=x128[:, s + o:s + o + n],
                        start=(t == 0),
                        stop=(t == 8),
                    )
                nc.scalar.activation(
                    out=xr2[:, j:j + n], in_=ps[:, :n], func=ID, bias=br[:, 0:1]
                )

            # ---- stage 1: children pairs ----
            rc_prev = None
            for p in range(4):
                cat = catt[p]
                rb = rb_pool.tile([2 * C, S1N], BF16, tag="rb")
                for s, n in S1WIN:
                    j = s - S1_LO
                    ps = psum_pool.tile([2 * C, NWIN], F32, tag="ps")
                    for t in range(9):
                        o = TAP_OFF[t]
                        nc.tensor.matmul(
                            out=ps[:, :n],
                            lhsT=wrc[:, t, :],
                            rhs=cat[:, s + o:s + o + n],
                            start=(t == 0),
                            stop=False,
                        )
                    nc.tensor.matmul(
                        out=ps[:, :n],
                        lhsT=i2h,
                        rhs=xr2[:, j:j + n],
                        start=False,
                        stop=True,
                    )
                    nc.scalar.activation(
                        out=rb[:, j:j + n], in_=ps[:, :n], func=SIG
                    )
                rc = rc_pool.tile([2 * C, S1N], BF16, tag="rc")
                nc.vector.tensor_mul(out=rc, in0=rb, in1=cat[:, S1_LO:S1_HI])
                if p == 1:
                    nc.vector.tensor_add(out=T, in0=rc_prev, in1=rc)
                elif p == 3:
                    nc.vector.tensor_add(out=rc, in0=rc_prev, in1=rc)
                    nc.vector.tensor_add(out=T, in0=T, in1=rc)
                rc_prev = rc
                # child_sum pair-tree rides the vector queue alongside
                if p == 1:
                    nc.vector.tensor_add(out=S, in0=catt[0], in1=catt[1])
                elif p == 2:
                    nc.vector.tensor_add(out=S, in0=S, in1=catt[2])
                elif p == 3:
                    nc.vector.tensor_add(out=S, in0=S, in1=catt[3])

            # ---- csum fold: zs[0:C] = S_low + S_high via [I;I] ----
            for s, n in S1WIN:
                ps = psumf_pool.tile([C, NWIN], F32, tag="psf")
                nc.tensor.matmul(out=ps[:, :n], lhsT=i2v, rhs=S[:, s:s + n])
                nc.scalar.activation(out=zs[0:C, s:s + n], in_=ps[:, :n], func=CP)

            # ---- z conv ----
            for s, n in S2WIN:
                j = s - S2_LO
                ps = psumf_pool.tile([C, NWIN], F32, tag="psf")
                for t in range(9):
                    o = TAP_OFF[t]
                    nc.tensor.matmul(
                        out=ps[:, :n],
                        lhsT=wz[:, t, :],
                        rhs=zs[:, s + o:s + o + n],
                        start=(t == 0),
                        stop=(t == 8),
                    )
                nc.scalar.activation(
                    out=zb[:, j:j + n], in_=ps[:, :n], func=SIG, bias=bz[:, 0:1]
                )

            # ---- rh fold: orh[0:C] = T_low + T_high ----
            for s, n in S1WIN:
                j = s - S1_LO
                ps = psumf_pool.tile([C, NWIN], F32, tag="psf")
                nc.tensor.matmul(out=ps[:, :n], lhsT=i2v, rhs=T[:, j:j + n])
                nc.scalar.activation(out=orh[0:C, s:s + n], in_=ps[:, :n], func=CP)

            # ---- o conv + h combine + store, per window ----
            for s, n in S2WIN:
                j = s - S2_LO
                ps = psumf_pool.tile([C, NWIN], F32, tag="psf")
                for t in range(9):
                    o = TAP_OFF[t]
                    nc.tensor.matmul(
                        out=ps[:, :n],
                        lhsT=wo[:, t, :],
                        rhs=orh[:, s + o:s + o + n],
                        start=(t == 0),
                        stop=(t == 8),
                    )
                nc.scalar.activation(
                    out=ob[:, j:j + n], in_=ps[:, :n], func=TANH, bias=bo[:, 0:1]
                )
                # h = o + z*(csum - o)
                t1 = hw_pool.tile([C, NWIN], BF16, tag="t1")
                nc.vector.scalar_tensor_tensor(
                    out=t1[:, :n],
                    in0=ob[:, j:j + n],
                    scalar=-1.0,
                    in1=zs[0:C, s:s + n],
                    op0=mybir.AluOpType.mult,
                    op1=mybir.AluOpType.add,
                )
                nc.vector.tensor_mul(out=t1[:, :n], in0=zb[:, j:j + n], in1=t1[:, :n])
                hst = hw_pool.tile([C, NWIN], BF16, tag="hst")
                nc.vector.tensor_add(out=hst[:, :n], in0=ob[:, j:j + n], in1=t1[:, :n])
                nc.sync.dma_start(out=hout[:, j:j + n], in_=hst[:, :n])

    nc.finalize()
    return nc


def _get_program():
    global _BUILT
    if _BUILT is None:
        _BUILT = build_program()
    return _BUILT


def make_in_maps(x, child_h, Wr, br, Wz, bz, Wo, bo):
    """Host-side sharding: pad borders/columns, slice 28-row slabs, bf16."""
    x = np.asarray(x, dtype=np.float32)
    child_h = np.asarray(child_h, dtype=np.float32)

    xp = np.zeros((C, HW + 4, WP), dtype=np.float32)
    xp[:, 2:2 + HW, 1:1 + HW] = x[0]
    cp = np.zeros((L, C, HW + 4, WP), dtype=np.float32)
    cp[:, :, 2:2 + HW, 1:1 + HW] = child_h[:, 0]

    def frame(a):  # [..., IN_ROWS, WP] -> [..., FREE] bf16 with front/tail pad
        flat = a.reshape(a.shape[:-2] + (FRAME,))
        out = np.zeros(a.shape[:-2] + (FREE,), dtype=NPBF16)
        out[..., 1:1 + FRAME] = flat.astype(NPBF16)
        return out

    def wt(w):  # [C, 2C, 3, 3] -> [2C(in), 9, C(out)]; in 0:C = x-half
        return np.transpose(np.asarray(w, np.float32), (1, 2, 3, 0)).reshape(2 * C, 9, C)

    wrt = wt(Wr)
    wrx = np.zeros((2 * C, 9, 2 * C), dtype=NPBF16)
    wrx[0:C, :, 0:C] = wrt[0:C].astype(NPBF16)
    wrx[0:C, :, C:2 * C] = wrt[0:C].astype(NPBF16)
    wrc = np.zeros((2 * C, 9, 2 * C), dtype=NPBF16)
    wrc[0:C, :, 0:C] = wrt[C:2 * C].astype(NPBF16)
    wrc[C:2 * C, :, C:2 * C] = wrt[C:2 * C].astype(NPBF16)

    def wswap(w):  # z/o lhsT with [csum/rh | x] partition order
        a = wt(w)
        return np.ascontiguousarray(
            np.concatenate([a[C:2 * C], a[0:C]], axis=0)
        ).astype(NPBF16)

    wzt, wot = wswap(Wz), wswap(Wo)
    idt = np.eye(2 * C).astype(NPBF16)
    idvt = np.concatenate([np.eye(C), np.eye(C)], axis=0).astype(NPBF16)
    brt = np.tile(np.asarray(br, np.float32).reshape(C, 1), (2, 1))
    bzt = np.asarray(bz, np.float32).reshape(C, 1)
    bot = np.asarray(bo, np.float32).reshape(C, 1)

    in_maps = []
    for k in range(NCORES):
        r0 = k * OUT_ROWS  # slab = global rows r0-2 .. r0+26
        in_maps.append({
            "xin": frame(xp[:, r0:r0 + IN_ROWS, :]),
            "cin": frame(cp[:, :, r0:r0 + IN_ROWS, :]),
            "wrxt": wrx, "wrct": wrc, "wzt": wzt, "wot": wot,
            "idt": idt, "idvt": idvt,
            "brt": brt, "bzt": bzt, "bot": bot,
        })
    return in_maps


def run(in_maps, trace=False):
    nc = _get_program()
    return run_bass_kernel_spmd(nc, in_maps, list(range(NCORES)), trace=trace)


def kernel(x, child_h, Wr, br, Wz, bz, Wo, bo):
    in_maps = make_in_maps(x, child_h, Wr, br, Wz, bz, Wo, bo)
    res = run(in_maps).results
    out = np.empty((1, C, HW, HW), dtype=np.float32)
    for k in range(NCORES):
        h = np.asarray(res[k]["hout"]).astype(np.float32)
        h = h.reshape(C, OUT_ROWS, WP)[:, :, 1:1 + HW]
        out[0, :, k * OUT_ROWS:(k + 1) * OUT_ROWS, :] = h
    return out


# revision 7
# speedup vs baseline: 1.9319x; 1.3550x over previous
"""ConvTreeGRUCell on 8 Trainium2 NeuronCores.

Sharding: spatial over H. Each core owns 24 output rows (192/8) and
receives a 28-row input slab (2-row halo each side, zero-padded at the
image borders on the host). All convs and L-reductions are local.

v2 (vs v1 bf16 restructure):
  - reset-gate convs (P0 x-part and the per-child-pair taps) run in
    fp8e4m3 DoubleRow mode: taps are paired into [K, 2, N] access
    patterns (second k-tile = the partner tap at column delta), so each
    DR matmul does 2 taps at 0.5 cycles/row.  Weights are scaled x32 on
    the host (keeps them out of the fp8 subnormal range); the
    activation un-scales with scale=1/32.  The xr2 inject uses a 32*I
    bf16 identity so the whole PSUM is uniformly scaled.
  - child_sum is accumulated exactly on the PE ([I;I] x cat_p, 4
    accumulating bf16 matmuls) and kept in f32 for the h-combine path
    (csum32); only the z-conv rhs copy is bf16.  This halves the
    elementwise error vs the bf16 sum tree.
  - z/o convs stay bf16 (fp8 on the big-magnitude csum input blows the
    error budget ~6e-2).
  - reset_hidden products accumulate window-wise into T (bf16) on
    Vector; cross-partition folds stay [I;I] matmuls.
"""

import os
import sys

import numpy as np
import ml_dtypes

for _p in ("/opt/trn_rl_repo",):
    if _p not in sys.path and os.path.isdir(_p):
        sys.path.insert(0, _p)

import concourse.bass as bass
import concourse.tile as tile
from concourse import bacc
from concourse import mybir
from concourse.ap import AP
from concourse.bass_utils import run_bass_kernel_spmd

F32 = mybir.dt.float32
BF16 = mybir.dt.bfloat16
FP8 = mybir.dt.float8e4
NPBF16 = ml_dtypes.bfloat16
NPFP8 = mybir.dt.np(FP8)
DR = mybir.MatmulPerfMode.DoubleRow
WSCALE = 32.0

C = 64          # channels
L = 8           # children
HW = 192        # image H and W
NCORES = 8
OUT_ROWS = HW // NCORES          # 24 output rows per core
IN_ROWS = OUT_ROWS + 4           # 28-row slab (2-row halo each side)
WP = HW + 2                      # 194: padded row width
FRAME = IN_ROWS * WP             # 5432
FREE = FRAME + 2                 # 5434: +1 front pad, +1 tail pad

# flat index of (row r, col c) in the frame = 1 + r*WP + c
S1_LO = 1 + 1 * WP               # 195   (r rows 1..26)
S1_HI = 1 + 26 * WP + 194        # 5239 (exclusive)
S1N = S1_HI - S1_LO              # 5044
S2_LO = 1 + 2 * WP               # 389   (h rows 2..25)
S2_HI = 1 + 25 * WP + 194        # 5045 (exclusive)
S2N = S2_HI - S2_LO              # 4656

NWIN = 512

TAP_OFF = [dy * WP + dx for dy in (-1, 0, 1) for dx in (-1, 0, 1)]
# DoubleRow tap pairs: (0,1) (2,3) (4,5) (6,7) (8,8-with-zero-weights)
DR_PAIRS = [(0, 1), (2, 3), (4, 5), (6, 7), (8, 8)]


def _windows(lo, hi):
    out = []
    s = lo
    while s < hi:
        out.append((s, min(NWIN, hi - s)))
        s += NWIN
    return out


S1WIN = _windows(S1_LO, S1_HI)
S2WIN = _windows(S2_LO, S2_HI)

_BUILT = None


def _dr_rhs(tile_ap, base_col, n, delta):
    """[K, 2, N] moving AP: k-tile 0 at base_col, k-tile 1 at +delta."""
    sl = tile_ap[:, base_col:base_col + n]
    dims = [list(d) for d in sl.ap]
    assert len(dims) == 2
    return AP(sl.tensor, sl.offset, [dims[0], [delta, 2], [1, n]])


def build_program():
    nc = bacc.Bacc("TRN2")

    x8t = nc.dram_tensor("x8t", [C, FREE], FP8, kind="ExternalInput")
    xin = nc.dram_tensor("xin", [C, FREE], BF16, kind="ExternalInput")
    cin8 = nc.dram_tensor("cin8", [L, C, FREE], FP8, kind="ExternalInput")
    cin = nc.dram_tensor("cin", [L, C, FREE], BF16, kind="ExternalInput")
    wrxt = nc.dram_tensor("wrxt", [2 * C, 5, 2, 2 * C], FP8, kind="ExternalInput")
    wrct = nc.dram_tensor("wrct", [2 * C, 5, 2, 2 * C], FP8, kind="ExternalInput")
    wzt = nc.dram_tensor("wzt", [2 * C, 9, C], BF16, kind="ExternalInput")
    wot = nc.dram_tensor("wot", [2 * C, 9, C], BF16, kind="ExternalInput")
    idt = nc.dram_tensor("idt", [2 * C, 2 * C], BF16, kind="ExternalInput")
    idvt = nc.dram_tensor("idvt", [2 * C, C], BF16, kind="ExternalInput")
    brt = nc.dram_tensor("brt", [2 * C, 1], F32, kind="ExternalInput")
    bzt = nc.dram_tensor("bzt", [C, 1], F32, kind="ExternalInput")
    bot = nc.dram_tensor("bot", [C, 1], F32, kind="ExternalInput")
    hout = nc.dram_tensor("hout", [C, S2N], BF16, kind="ExternalOutput")

    ID = mybir.ActivationFunctionType.Identity
    SIG = mybir.ActivationFunctionType.Sigmoid
    TANH = mybir.ActivationFunctionType.Tanh
    CP = mybir.ActivationFunctionType.Copy
    INV = 1.0 / WSCALE

    with tile.TileContext(nc) as tc:
        with (
            tc.tile_pool(name="singles", bufs=1) as singles,
            tc.tile_pool(name="cats", bufs=1) as cats,
            tc.tile_pool(name="rbp", bufs=3) as rb_pool,
            tc.tile_pool(name="hwp", bufs=3) as hw_pool,
            tc.tile_pool(name="psum", bufs=4, space="PSUM") as psum_pool,
            tc.tile_pool(name="psumf", bufs=4, space="PSUM") as psumf_pool,
        ):
            # ---- persistent tiles ----
            x8 = singles.tile([2 * C, FREE], FP8, tag="x8")
            wrx = singles.tile([2 * C, 5, 2, 2 * C], FP8, tag="wrx")
            wrc = singles.tile([2 * C, 5, 2, 2 * C], FP8, tag="wrc")
            wz = singles.tile([2 * C, 9, C], BF16, tag="wz")
            wo = singles.tile([2 * C, 9, C], BF16, tag="wo")
            i2h = singles.tile([2 * C, 2 * C], BF16, tag="i2h")   # 32*I
            i2v = singles.tile([2 * C, C], BF16, tag="i2v")       # [I;I]
            br = singles.tile([2 * C, 1], F32, tag="br")
            bz = singles.tile([C, 1], F32, tag="bz")
            bo = singles.tile([C, 1], F32, tag="bo")
            xr2 = singles.tile([2 * C, S1N], BF16, tag="xr2")
            zs = singles.tile([2 * C, FREE], BF16, tag="zs")      # [csum | x]
            orh = singles.tile([2 * C, FREE], BF16, tag="orh")    # [rh | x]
            csum32 = singles.tile([C, S2N], F32, tag="csum32")
            T = singles.tile([2 * C, S1N], BF16, tag="T")         # sum r*child
            zb = singles.tile([C, S2N], BF16, tag="zb")
            ob = singles.tile([C, S2N], BF16, tag="ob")

            # ---- loads: matmul-critical order ----
            nc.sync.dma_start(out=x8[0:C, :], in_=x8t[:])
            nc.sync.dma_start(out=x8[C:2 * C, :], in_=x8t[:])
            nc.sync.dma_start(out=wrx, in_=wrxt[:])
            nc.sync.dma_start(out=br, in_=brt[:])
            nc.sync.dma_start(out=wrc, in_=wrct[:])
            nc.sync.dma_start(out=i2h, in_=idt[:])
            cat8t, catt = [], []
            for p in range(4):
                c8 = cats.tile([2 * C, FREE], FP8, tag=f"cat8_{p}")
                nc.sync.dma_start(out=c8[0:C, :], in_=cin8[2 * p])
                nc.sync.dma_start(out=c8[C:2 * C, :], in_=cin8[2 * p + 1])
                cat8t.append(c8)
                cb = cats.tile([2 * C, FREE], BF16, tag=f"cat{p}")
                nc.sync.dma_start(out=cb[0:C, :], in_=cin[2 * p])
                nc.sync.dma_start(out=cb[C:2 * C, :], in_=cin[2 * p + 1])
                catt.append(cb)
            nc.sync.dma_start(out=i2v, in_=idvt[:])
            nc.sync.dma_start(out=wz, in_=wzt[:])
            nc.sync.dma_start(out=wo, in_=wot[:])
            nc.sync.dma_start(out=bz, in_=bzt[:])
            nc.sync.dma_start(out=bo, in_=bot[:])
            nc.sync.dma_start(out=zs[C:2 * C, :], in_=xin[:])
            nc.sync.dma_start(out=orh[C:2 * C, :], in_=xin[:])
            # zero the csum/rh halves (pad cols outside S1 must be 0)
            nc.scalar.memzero(zs[0:C, :])
            nc.scalar.memzero(orh[0:C, :])

            # ---- P0: xr2 = [Wr_x*x + br] (x2 on halves), fp8 DoubleRow ----
            for s, n in S1WIN:
                j = s - S1_LO
                ps = psum_pool.tile([2 * C, NWIN], F32, tag="ps")
                for i, (ta, tb) in enumerate(DR_PAIRS):
                    oa = TAP_OFF[ta]
                    nc.tensor.matmul(
                        out=ps[:, :n],
                        lhsT=wrx[:, i, :, :],
                        rhs=_dr_rhs(x8, s + oa, n, TAP_OFF[tb] - oa),
                        start=(i == 0),
                        stop=(i == 4),
                        perf_mode=DR,
                    )
                nc.scalar.activation(
                    out=xr2[:, j:j + n], in_=ps[:, :n], func=ID,
                    bias=br[:, 0:1], scale=INV,
                )

            # ---- stage 1: children pairs (fp8 DR taps + bf16 inject) ----
            for p in range(4):
                c8 = cat8t[p]
                cb = catt[p]
                for s, n in S1WIN:
                    j = s - S1_LO
                    ps = psum_pool.tile([2 * C, NWIN], F32, tag="ps")
                    for i, (ta, tb) in enumerate(DR_PAIRS):
                        oa = TAP_OFF[ta]
                        nc.tensor.matmul(
                            out=ps[:, :n],
                            lhsT=wrc[:, i, :, :],
                            rhs=_dr_rhs(c8, s + oa, n, TAP_OFF[tb] - oa),
                            start=(i == 0),
                            stop=False,
                            perf_mode=DR,
                        )
                    nc.tensor.matmul(
                        out=ps[:, :n], lhsT=i2h, rhs=xr2[:, j:j + n],
                        start=False, stop=True,
                    )
                    rb = rb_pool.tile([2 * C, NWIN], BF16, tag="rb")
                    nc.scalar.activation(
                        out=rb[:, :n], in_=ps[:, :n], func=SIG, scale=INV,
                    )
                    # T[:, w] (+)= rb * child  (bf16, 128 partitions)
                    if p == 0:
                        nc.vector.tensor_mul(
                            out=T[:, j:j + n], in0=rb[:, :n], in1=cb[:, s:s + n]
                        )
                    else:
                        tm = rb_pool.tile([2 * C, NWIN], BF16, tag="tm")
                        nc.vector.tensor_mul(
                            out=tm[:, :n], in0=rb[:, :n], in1=cb[:, s:s + n]
                        )
                        nc.vector.tensor_add(
                            out=T[:, j:j + n], in0=T[:, j:j + n], in1=tm[:, :n]
                        )

            # ---- csum: PE-accumulated exact sum of all 8 children ----
            for s, n in S1WIN:
                ps = psumf_pool.tile([C, NWIN], F32, tag="psf")
                for p in range(4):
                    nc.tensor.matmul(
                        out=ps[:, :n], lhsT=i2v, rhs=catt[p][:, s:s + n],
                        start=(p == 0), stop=(p == 3),
                    )
                nc.scalar.activation(out=zs[0:C, s:s + n], in_=ps[:, :n], func=CP)
                ov_lo, ov_hi = max(s, S2_LO), min(s + n, S2_HI)
                if ov_lo < ov_hi:
                    nc.scalar.activation(
                        out=csum32[:, ov_lo - S2_LO:ov_hi - S2_LO],
                        in_=ps[:, ov_lo - s:ov_hi - s], func=CP,
                    )

            # ---- z conv (bf16) ----
            for s, n in S2WIN:
                j = s - S2_LO
                ps = psumf_pool.tile([C, NWIN], F32, tag="psf")
                for t in range(9):
                    o = TAP_OFF[t]
                    nc.tensor.matmul(
                        out=ps[:, :n],
                        lhsT=wz[:, t, :],
                        rhs=zs[:, s + o:s + o + n],
                        start=(t == 0),
                        stop=(t == 8),
                    )
                nc.scalar.activation(
                    out=zb[:, j:j + n], in_=ps[:, :n], func=SIG, bias=bz[:, 0:1]
                )

            # ---- rh fold: orh[0:C] = T_low + T_high ----
            for s, n in S1WIN:
                j = s - S1_LO
                ps = psumf_pool.tile([C, NWIN], F32, tag="psf")
                nc.tensor.matmul(out=ps[:, :n], lhsT=i2v, rhs=T[:, j:j + n])
                nc.scalar.activation(out=orh[0:C, s:s + n], in_=ps[:, :n], func=CP)

            # ---- o conv + h combine + store, per window ----
            for s, n in S2WIN:
                j = s - S2_LO
                ps = psumf_pool.tile([C, NWIN], F32, tag="psf")
                for t in range(9):
                    o = TAP_OFF[t]
                    nc.tensor.matmul(
                        out=ps[:, :n],
                        lhsT=wo[:, t, :],
                        rhs=orh[:, s + o:s + o + n],
                        start=(t == 0),
                        stop=(t == 8),
                    )
                nc.scalar.activation(
                    out=ob[:, j:j + n], in_=ps[:, :n], func=TANH, bias=bo[:, 0:1]
                )
                # h = o + z*(csum - o), csum path in f32
                t1 = hw_pool.tile([C, NWIN], F32, tag="t1")
                nc.vector.scalar_tensor_tensor(
                    out=t1[:, :n],
                    in0=ob[:, j:j + n],
                    scalar=-1.0,
                    in1=csum32[:, j:j + n],
                    op0=mybir.AluOpType.mult,
                    op1=mybir.AluOpType.add,
                )
                nc.vector.tensor_mul(out=t1[:, :n], in0=zb[:, j:j + n], in1=t1[:, :n])
                hst = hw_pool.tile([C, NWIN], BF16, tag="hst")
                nc.vector.tensor_add(out=hst[:, :n], in0=ob[:, j:j + n], in1=t1[:, :n])
                nc.sync.dma_start(out=hout[:, j:j + n], in_=hst[:, :n])

    nc.finalize()
    return nc


def _get_program():
    global _BUILT
    if _BUILT is None:
        _BUILT = build_program()
    return _BUILT


def make_in_maps(x, child_h, Wr, br, Wz, bz, Wo, bo):
    """Host-side sharding: pad borders/columns, slice 28-row slabs."""
    x = np.asarray(x, dtype=np.float32)
    child_h = np.asarray(child_h, dtype=np.float32)

    xp = np.zeros((C, HW + 4, WP), dtype=np.float32)
    xp[:, 2:2 + HW, 1:1 + HW] = x[0]
    cp = np.zeros((L, C, HW + 4, WP), dtype=np.float32)
    cp[:, :, 2:2 + HW, 1:1 + HW] = child_h[:, 0]

    def frame(a, dt):  # [..., IN_ROWS, WP] -> [..., FREE] with front/tail pad
        flat = a.reshape(a.shape[:-2] + (FRAME,))
        out = np.zeros(a.shape[:-2] + (FREE,), dtype=dt)
        out[..., 1:1 + FRAME] = flat.astype(dt)
        return out

    def wt(w):  # [C, 2C, 3, 3] -> [2C(in), 9, C(out)]; in 0:C = x-half
        return np.transpose(np.asarray(w, np.float32), (1, 2, 3, 0)).reshape(2 * C, 9, C)

    def drpack(w64, rows):
        """w64: [C(in), 9, C(out)] x-or-child half -> [2C, 5, 2, 2C] fp8 x32.
        rows: (row offset pairs) describing where the in-channels sit for
        each output half; here we place per spec below."""
        out = np.zeros((2 * C, 5, 2, 2 * C), dtype=np.float32)
        for i, (ta, tb) in enumerate(DR_PAIRS):
            for k, tap in ((0, ta), (1, tb)):
                if i == 4 and k == 1:
                    continue  # zero weights pair up the lone 9th tap
                for (rlo, clo) in rows:
                    out[rlo:rlo + C, i, k, clo:clo + C] = w64[:, tap, :]
        return (out * WSCALE).astype(NPFP8)

    wrt = wt(Wr)
    # P0: x channels on partitions 0:C (and a copy of x on C:2C that gets
    # zero weights); outputs [xr | xr] -> weight blocks (0,0) and (0,C)
    wrx = drpack(wrt[0:C], [(0, 0), (0, C)])
    # pairs: block-diag child weights
    wrc = drpack(wrt[C:2 * C], [(0, 0), (C, C)])

    def wswap(w):  # z/o lhsT with [csum/rh | x] partition order
        a = wt(w)
        return np.ascontiguousarray(
            np.concatenate([a[C:2 * C], a[0:C]], axis=0)
        ).astype(NPBF16)

    wzt, wot = wswap(Wz), wswap(Wo)
    idt = (WSCALE * np.eye(2 * C)).astype(NPBF16)
    idvt = np.concatenate([np.eye(C), np.eye(C)], axis=0).astype(NPBF16)
    brt = np.tile(np.asarray(br, np.float32).reshape(C, 1), (2, 1))
    bzt = np.asarray(bz, np.float32).reshape(C, 1)
    bot = np.asarray(bo, np.float32).reshape(C, 1)

    in_maps = []
    for k in range(NCORES):
        r0 = k * OUT_ROWS  # slab = global rows r0-2 .. r0+26
        xs = xp[:, r0:r0 + IN_ROWS, :]
        cs = cp[:, :, r0:r0 + IN_ROWS, :]
        in_maps.append({
            "x8t": frame(xs, NPFP8), "xin": frame(xs, NPBF16),
            "cin8": frame(cs, NPFP8), "cin": frame(cs, NPBF16),
            "wrxt": wrx, "wrct": wrc, "wzt": wzt, "wot": wot,
            "idt": idt, "idvt": idvt,
            "brt": brt, "bzt": bzt, "bot": bot,
        })
    return in_maps


def run(in_maps, trace=False):
    nc = _get_program()
    return run_bass_kernel_spmd(nc, in_maps, list(range(NCORES)), trace=trace)


def kernel(x, child_h, Wr, br, Wz, bz, Wo, bo):
    in_maps = make_in_maps(x, child_h, Wr, br, Wz, bz, Wo, bo)
    res = run(in_maps).results
    out = np.empty((1, C, HW, HW), dtype=np.float32)
    for k in range(NCORES):
        h = np.asarray(res[k]["hout"]).astype(np.float32)
        h = h.reshape(C, OUT_ROWS, WP)[:, :, 1:1 + HW]
        out[0, :, k * OUT_ROWS:(k + 1) * OUT_ROWS, :] = h
    return out
